# revision 12
# baseline (speedup 1.0000x reference)
"""CRF loss (forward-algorithm log-partition + joint score) on 8 TRN2 cores.

Sharding: pure data parallel. 256 batch rows -> 8 cores x 32 rows.

Per core, exp-domain forward recursion with emissions centered by a constant
(exp(x - CEN)) so the state magnitude stays O(1) for the whole sequence --
no mid-chain renormalization needed (ln colsum drifts within [-7, +10] vs
bf16's +-88).  The serial chain is split in half: a forward recursion from
t=0 and a backward recursion from t=1023 run as two independent
matmul->multiply chains interleaved on PE/DVE, meeting at t=511 where
Z_b = sum_j (W^T alpha_511)[j] * B_512[j].

Layout: host pre-transposes emissions to (97 tags, 1024*32 cols) time-major
so every DMA is contiguous per partition.  ACT exp's the staged f32 chunks
to bf16 X tiles for the recursion and Copy's them to bf16 Mb tiles for the
numerator.  The joint-score numerator uses a host-built bf16 one-hot of the
tags in the same layout, fully on PE via accumulating block matmuls:
diag(O_blk^T Mb_blk) gives emission scores, diag(Oshift_blk^T TPb_blk) with
TP = trans^T O gives transition scores; all 512 block products accumulate
into one [128,128] PSUM tile whose diagonal is extracted once at the end.
Start/end scores are two [32,1] matmuls.  No gathers, no GPSIMD compute, no
PE transposes, contiguous DMA only, ~2.6k instructions.

Host folds back: loss = sum(acc2 slots) - sum(ln z) - BL*S*CEN per core.
"""

import numpy as np
import ml_dtypes

import concourse.bacc as bacc
import concourse.bass as bass
import concourse.mybir as mybir
import concourse.tile as tile
from concourse import bass_utils, masks

B, S, T = 256, 1024, 97
NCORES = 8
BL = B // NCORES          # 32 batch rows per core
SC = 64                   # timesteps per super-chunk
SCC = SC * BL             # 2048 columns per super-chunk
NSC = S // SC             # 16 super-chunks
TPC = 512                 # columns per transition-score matmul (one PSUM bank)
DBL = 128                 # columns per diagonal-trick block matmul
CEN = 5.07                # exp-domain centering constant
MEET = S // 2 - 1         # 511: forward steps 1..511, backward 1022..512
OPAD = 64                 # one-hot column padding (shifted reads + last tile)

F32 = mybir.dt.float32
BF16 = mybir.dt.bfloat16
ALU = mybir.AluOpType
AXX = mybir.AxisListType
ACT = mybir.ActivationFunctionType


def build_module(with_numerator=True, with_recursion=True, drain=2,
                 ebufs=4, pbufs=4, tpbufs=2, order="ffbb"):
    nc = bacc.Bacc("TRN2", target_bir_lowering=False, debug=False)

    xT_d = nc.dram_tensor("xT_d", [T, S * BL], F32, kind="ExternalInput").ap()
    oh_d = nc.dram_tensor("oh_d", [T, S * BL + OPAD], BF16,
                          kind="ExternalInput").ap()
    tr_d = nc.dram_tensor("tr_d", [T, T], F32, kind="ExternalInput").ap()
    trT_d = nc.dram_tensor("trT_d", [T, T], F32, kind="ExternalInput").ap()
    start_d = nc.dram_tensor("start_d", [T, 1], F32, kind="ExternalInput").ap()
    end_d = nc.dram_tensor("end_d", [T, 1], F32, kind="ExternalInput").ap()
    z_d = nc.dram_tensor("z_d", [1, BL], F32, kind="ExternalOutput").ap()
    acc2_d = nc.dram_tensor("acc2_d", [128, 2], F32, kind="ExternalOutput").ap()

    with tile.TileContext(nc) as tc:
        with (
            tc.tile_pool(name="const", bufs=1) as const_pool,
            tc.tile_pool(name="stage", bufs=4) as stage_pool,
            tc.tile_pool(name="xpool", bufs=8) as x_pool,
            tc.tile_pool(name="opool", bufs=4) as o_pool,
            tc.tile_pool(name="mb", bufs=2) as mb_pool,
            tc.tile_pool(name="tpb", bufs=2) as tpb_pool,
            tc.tile_pool(name="state", bufs=ebufs) as e_pool,
            tc.tile_pool(name="pp", bufs=pbufs, space=bass.MemorySpace.PSUM) as p_pool,
            tc.tile_pool(name="tp", bufs=tpbufs, space=bass.MemorySpace.PSUM) as tp_pool,
            tc.tile_pool(name="dacc", bufs=1, space=bass.MemorySpace.PSUM) as dacc_pool,
            tc.tile_pool(name="cs", bufs=1, space=bass.MemorySpace.PSUM) as cs_pool,
        ):
            # ---------------- constants ----------------
            tr_stage = const_pool.tile([T, T], F32)
            nc.sync.dma_start(tr_stage[:], tr_d[:, :])
            W = const_pool.tile([T, T], BF16)
            nc.scalar.activation(W[:], tr_stage[:], ACT.Exp)
            tr_bf = const_pool.tile([T, T], BF16)
            nc.scalar.activation(tr_bf[:], tr_stage[:], ACT.Copy)

            trT_stage = const_pool.tile([T, T], F32)
            nc.sync.dma_start(trT_stage[:], trT_d[:, :])
            WT = const_pool.tile([T, T], BF16)
            nc.scalar.activation(WT[:], trT_stage[:], ACT.Exp)

            st_stage = const_pool.tile([T, 1], F32)
            nc.sync.dma_start(st_stage[:], start_d[:, :])
            exp_start = const_pool.tile([T, 1], F32)
            nc.scalar.activation(exp_start[:], st_stage[:], ACT.Exp)
            start_bf = const_pool.tile([T, 1], BF16)
            nc.scalar.activation(start_bf[:], st_stage[:], ACT.Copy)

            en_stage = const_pool.tile([T, 1], F32)
            nc.sync.dma_start(en_stage[:], end_d[:, :])
            exp_end = const_pool.tile([T, 1], F32)
            nc.scalar.activation(exp_end[:], en_stage[:], ACT.Exp)
            end_bf = const_pool.tile([T, 1], BF16)
            nc.scalar.activation(end_bf[:], en_stage[:], ACT.Copy)

            ones_col = const_pool.tile([T, 1], BF16)
            nc.vector.memset(ones_col[:], 1.0)
            cen_bias = const_pool.tile([T, 1], F32)
            nc.vector.memset(cen_bias[:], -CEN)
            ident = const_pool.tile([128, 128], F32)
            masks.make_identity(nc, ident[:])
            ones32 = const_pool.tile([BL, 1], F32)
            nc.vector.memset(ones32[:], 1.0)

            acc2 = const_pool.tile([128, 2], F32)
            nc.vector.memset(acc2[:], 0.0)

            diagacc = None
            if with_numerator:
                diagacc = dacc_pool.tile([128, 128], F32, tag="dacc")

            xsc = [None] * NSC
            pend = []          # deferred diag-block matmul closures
            NDIAG = 2 * NSC * (SCC // DBL)   # 512 block matmuls in the group
            state = {"ndone": 0}

            def diag_mm(lhs_ap, rhs_ap, n):
                def emit():
                    i = state["ndone"]
                    state["ndone"] = i + 1
                    nc.tensor.matmul(diagacc[0:n, 0:n], lhs_ap, rhs_ap,
                                     start=(i == 0), stop=(i == NDIAG - 1),
                                     skip_group_check=True)
                pend.append(emit)

            # ------------- super-chunk producer + numerator -------------
            def produce(k):
                c0 = k * SCC
                st = stage_pool.tile([T, SCC], F32, tag="stage")
                nc.sync.dma_start(st[:], xT_d[:, c0:c0 + SCC])
                xc = x_pool.tile([T, SCC], BF16, tag="X")
                nc.scalar.activation(xc[:], st[:], ACT.Exp, bias=cen_bias[:])
                xsc[k] = xc

                oh = o_pool.tile([T, SCC + BL], BF16, tag="O")
                nc.sync.dma_start(oh[:], oh_d[:, c0:c0 + SCC + BL])

                if not with_numerator:
                    return xc
                mb = mb_pool.tile([T, SCC], BF16, tag="mb")
                nc.scalar.activation(mb[:], st[:], ACT.Copy)

                tpb = tpb_pool.tile([T, SCC], BF16, tag="tpb")
                for c in range(SCC // TPC):
                    tp = tp_pool.tile([T, TPC], F32, tag="tp")
                    nc.tensor.matmul(tp[:], tr_bf[:],
                                     oh[:, c * TPC:(c + 1) * TPC])
                    nc.scalar.activation(tpb[:, c * TPC:(c + 1) * TPC], tp[:],
                                         ACT.Copy)

                # emission scores: diag(O_blk^T Mb_blk), PSUM-accumulated
                for g in range(SCC // DBL):
                    diag_mm(oh[:, g * DBL:(g + 1) * DBL],
                            mb[:, g * DBL:(g + 1) * DBL], DBL)
                # transition scores: diag(Oshift_blk^T TPb_blk)
                ncols = SCC if k < NSC - 1 else SCC - BL
                for g in range((ncols + DBL - 1) // DBL):
                    n = min(DBL, ncols - g * DBL)
                    diag_mm(oh[:, BL + g * DBL:BL + g * DBL + n],
                            tpb[:, g * DBL:g * DBL + n], n)

                if k == 0:
                    se = cs_pool.tile([BL, 1], F32, tag="se")
                    nc.tensor.matmul(se[:], oh[:, 0:BL], start_bf[:],
                                     start=True, stop=False,
                                     skip_group_check=True)
                    state["se"] = (se, oh)
                if k == NSC - 1:
                    se, _ = state["se"]
                    nc.tensor.matmul(se[:], oh[:, SCC - BL:SCC], end_bf[:],
                                     start=False, stop=True,
                                     skip_group_check=True)
                    dse = const_pool.tile([BL, 1], F32)
                    nc.vector.scalar_tensor_tensor(
                        dse[:], se[:], 1.0, ones32[:], ALU.mult, ALU.mult,
                        accum_out=acc2[0:BL, 1:2])
                return xc

            produce(0)
            produce(NSC - 1)
            produce(1)
            produce(NSC - 2)

            # ---------------- init both chains ----------------
            e_f = e_pool.tile([T, BL], BF16, tag="E")
            nc.vector.tensor_scalar_mul(e_f[:], xsc[0][:, 0:BL], exp_start[:])
            e_b = e_pool.tile([T, BL], BF16, tag="E")
            nc.vector.tensor_scalar_mul(e_b[:], xsc[NSC - 1][:, SCC - BL:SCC],
                                        exp_end[:])

            # ---------------- interleaved fwd/bwd recursion ----------------
            for s in range(1, MEET + 1):
                tf = s
                tb = (S - 1) - s
                kf, jf = divmod(tf, SC)
                kb, jb = divmod(tb, SC)
                if jf == 16 and kf + 2 <= NSC // 2 - 1:
                    produce(kf + 2)
                if jb == 47 and kb - 2 >= NSC // 2:
                    produce(kb - 2)

                if with_recursion:
                    if order == "ffbb":
                        pf = p_pool.tile([T, BL], F32, tag="P")
                        nc.tensor.matmul(pf[:], W[:], e_f[:])
                        pb = p_pool.tile([T, BL], F32, tag="P")
                        nc.tensor.matmul(pb[:], WT[:], e_b[:])
                        ef_new = e_pool.tile([T, BL], BF16, tag="E")
                        nc.vector.tensor_tensor(
                            ef_new[:], pf[:],
                            xsc[kf][:, jf * BL:(jf + 1) * BL], ALU.mult)
                        eb_new = e_pool.tile([T, BL], BF16, tag="E")
                        nc.vector.tensor_tensor(
                            eb_new[:], pb[:],
                            xsc[kb][:, jb * BL:(jb + 1) * BL], ALU.mult)
                    else:  # "fbfb": mm_f, mult_f, mm_b, mult_b
                        pf = p_pool.tile([T, BL], F32, tag="P")
                        nc.tensor.matmul(pf[:], W[:], e_f[:])
                        ef_new = e_pool.tile([T, BL], BF16, tag="E")
                        nc.vector.tensor_tensor(
                            ef_new[:], pf[:],
                            xsc[kf][:, jf * BL:(jf + 1) * BL], ALU.mult)
                        pb = p_pool.tile([T, BL], F32, tag="P")
                        nc.tensor.matmul(pb[:], WT[:], e_b[:])
                        eb_new = e_pool.tile([T, BL], BF16, tag="E")
                        nc.vector.tensor_tensor(
                            eb_new[:], pb[:],
                            xsc[kb][:, jb * BL:(jb + 1) * BL], ALU.mult)
                    e_f, e_b = ef_new, eb_new

                for _ in range(drain):
                    if pend:
                        pend.pop(0)()

            while pend:
                pend.pop(0)()

            # ---------------- meet in the middle ----------------
            pstar = p_pool.tile([T, BL], F32, tag="P")
            nc.tensor.matmul(pstar[:], W[:], e_f[:])
            zt = e_pool.tile([T, BL], BF16, tag="E")
            nc.vector.tensor_tensor(zt[:], pstar[:], e_b[:], ALU.mult)
            cs = cs_pool.tile([1, BL], F32, tag="se")
            nc.tensor.matmul(cs[:], ones_col[:], zt[:])
            zs = const_pool.tile([1, BL], F32)
            nc.vector.tensor_copy(zs[:], cs[:])
            nc.sync.dma_start(z_d[:, :], zs[:])

            # numerator: extract the accumulated diagonal
            if with_numerator:
                dumd = const_pool.tile([128, 128], F32)
                nc.vector.scalar_tensor_tensor(
                    dumd[:], diagacc[:], 1.0, ident[:], ALU.mult, ALU.mult,
                    accum_out=acc2[:, 0:1])
            nc.sync.dma_start(acc2_d[:, :], acc2[:])

    nc.compile()
    return nc


_cached = {}


def kernel(inputs, transitions, start_transitions, end_transitions, tags, mask):
    inputs = np.ascontiguousarray(np.asarray(inputs, dtype=np.float32))
    tags = np.ascontiguousarray(np.asarray(tags, dtype=np.int32))
    transitions = np.ascontiguousarray(np.asarray(transitions, dtype=np.float32))
    start = np.asarray(start_transitions, dtype=np.float32).reshape(T, 1)
    end = np.asarray(end_transitions, dtype=np.float32).reshape(T, 1)

    if "nc" not in _cached:
        _cached["nc"] = build_module()
    nc = _cached["nc"]

    transT = np.ascontiguousarray(transitions.T)
    tag_iota = np.arange(T, dtype=np.int32)[:, None]
    one_bits = np.uint16(0x3F80)  # bf16 1.0

    in_maps = []
    for c in range(NCORES):
        sl = slice(c * BL, (c + 1) * BL)
        xT = np.ascontiguousarray(
            inputs[sl].transpose(2, 1, 0).reshape(T, S * BL))
        flat = tags[sl].T.reshape(1, S * BL)  # time-major (t*BL + b)
        oh16 = np.zeros((T, S * BL + OPAD), dtype=np.uint16)
        oh16[:, :S * BL] = np.where(flat == tag_iota, one_bits, np.uint16(0))
        oh = oh16.view(ml_dtypes.bfloat16)
        in_maps.append({
            "xT_d": xT,
            "oh_d": oh,
            "tr_d": transitions,
            "trT_d": transT,
            "start_d": np.ascontiguousarray(start),
            "end_d": np.ascontiguousarray(end),
        })

    res = bass_utils.run_bass_kernel_spmd(nc, in_maps,
                                          core_ids=list(range(NCORES)))
    _cached["last_results"] = res
    _cached["last_in_maps"] = in_maps

    loss = np.float64(0.0)
    for c in range(NCORES):
        out = res.results[c]
        z = np.asarray(out["z_d"], dtype=np.float64).reshape(BL)
        a2 = np.asarray(out["acc2_d"], dtype=np.float64)
        loss += (a2[:, 0].sum() + a2[0:BL, 1].sum()
                 - np.log(z).sum() - BL * S * np.float64(CEN))
    return np.float32(loss)


def bench_exec(iters=20):
    """Time repeated executions of the compiled NEFF with device-resident
    inputs (mirrors bass2jax.run_bass_via_pjrt's multi-core path, minus
    donation so the jitted fn can be re-invoked)."""
    import time

    import jax
    import numpy as jnp_np
    from jax.sharding import Mesh, NamedSharding, PartitionSpec
    from jax.experimental.shard_map import shard_map

    from concourse import bass2jax as b2j
    import concourse.mybir as mybir_

    nc = _cached["nc"]
    in_maps = _cached["last_in_maps"]
    b2j.install_neuronx_cc_hook()

    partition_name = nc.partition_id_tensor.name if nc.partition_id_tensor else None
    in_names, out_names, out_avals, zero_outs = [], [], [], []
    for alloc in nc.m.functions[0].allocations:
        if not isinstance(alloc, mybir_.MemoryLocationSet):
            continue
        name = alloc.memorylocations[0].name
        if alloc.kind == "ExternalInput":
            if name != partition_name:
                in_names.append(name)
        elif alloc.kind == "ExternalOutput":
            shape = tuple(alloc.tensor_shape)
            dtype = mybir_.dt.np(alloc.dtype)
            out_avals.append(jax.core.ShapedArray(shape, dtype))
            zero_outs.append(np.zeros(shape, dtype))
            out_names.append(name)
    n_params = len(in_names)
    all_in = list(in_names) + list(out_names)
    if partition_name is not None:
        all_in.append(partition_name)

    def _body(*args):
        operands = list(args)
        if partition_name is not None:
            operands.append(b2j.partition_id_tensor())
        outs = b2j._bass_exec_p.bind(
            *operands, out_avals=tuple(out_avals), in_names=tuple(all_in),
            out_names=tuple(out_names), lowering_input_output_aliases=(),
            sim_require_finite=True, sim_require_nnan=True, nc=nc)
        return tuple(outs)

    devices = jax.devices()[:NCORES]
    mesh = Mesh(jnp_np.asarray(devices), ("core",))
    spec = PartitionSpec("core")
    n_outs = len(out_avals)
    fn = jax.jit(shard_map(_body, mesh=mesh, in_specs=(spec,) * (n_params + n_outs),
                           out_specs=(spec,) * n_outs, check_rep=False),
                 keep_unused=True)
    sh = NamedSharding(mesh, spec)
    concat_in = [
        jax.device_put(np.concatenate([np.asarray(in_maps[c][nm]) for c in range(NCORES)], axis=0), sh)
        for nm in in_names
    ]
    concat_zeros = [
        jax.device_put(np.zeros((NCORES * z.shape[0], *z.shape[1:]), z.dtype), sh)
        for z in zero_outs
    ]
    outs = fn(*concat_in, *concat_zeros)  # warmup/compile
    jax.block_until_ready(outs)
    times = []
    for _ in range(iters):
        t0 = time.perf_counter()
        outs = fn(*concat_in, *concat_zeros)
        jax.block_until_ready(outs)
        times.append(time.perf_counter() - t0)
    return min(times), sorted(times)[len(times) // 2], outs, out_names
